# revision 24
# baseline (speedup 1.0000x reference)
"""Trainium2 Bass kernel for BinOverlapPredictionFromMaxProj (segment max + masked mean).

Full computation:
  ptm: (32, 8, 30, 1, 72, 72) f32, mem_mask: (32, 8, 30) bool
  n = 32*8 = 256 rows; per row: max over 5184-feature axis per mem (30), then
  masked mean over mems -> out (256,) f32.

Sharding: data-parallel over the 256 fused rows across 8 cores (32 rows each).
Per core: 960 segments x 5184 features (~19.9 MB) -> memory-bound.

Device plan per core (raw bass, no TileContext): the shard's flat stream is
viewed as (128 partitions, 60, 648) -- each partition row holds 15 aligned
half-segments of 2592 floats = 60 QUARTER-columns of 648 floats.
  - Streaming loads on the gpsimd SWDGE queue. SWDGE's completion semaphore
    is WAW-ordered behind the data writes (HWDGE's then_inc was measured
    firing up to ~10us before the bytes actually landed in SBUF -- unusable
    as a data-ready signal mid-stream -- and concurrent HWDGE writes also
    slowed DVE reduces ~20%).
  - Chunks geometrically DECREASE (19,13,9,6,4,3,2,1,1,1,1 qcols): DVE
    reduce costs ~0.69x the DMA time per byte, so sizes shrinking by >=~0.7
    let the vector engine drain its backlog and finish the last chunk's
    reduce right after the last DMA byte lands.
  - All reduce_max on vector (only DVE has free-axis reduce) into
    qstats (128, 60); one cheap join-reduce folds to (128, 15).
  - One small SBUF->SBUF DMA repartitions the 1920 half-maxes (128, 15) ->
    (32, 60) (both APs walk half-segments in ascending order); then TWO
    fused scalar_tensor_tensor ops finish: pairwise max of the half pairs,
    then (mx * rcnt) * mask with accum_out giving the row sum directly.
  - The DVE pipeline has no same-engine RAW interlock (a dependent op can
    sample inputs before the previous op's write commits), so dependent
    same-engine pairs are separated by tsem ticks.

Raw bass instead of TileContext: Tile's drain/barrier/sem-free teardown adds
~2-3us extra inside the measured window, and Tile's scheduler is unneeded
for this fixed pipeline. The NEFF wrapper's own epilogue (per-sem zeroing
spam, ~8us) is emitted either way; nothing in-kernel can remove it.
"""

import sys

import numpy as np

if "/opt/trn_rl_repo" not in sys.path:
    sys.path.insert(0, "/opt/trn_rl_repo")

NCORES = 8
NF, NS, NMEM, FEAT = 32, 8, 30, 5184
N = NF * NS  # 256
ROWS = N // NCORES  # 32 rows per core
SEGS = ROWS * NMEM  # 960 segments per core
PPART = 128  # partitions
HALF = FEAT // 2  # 2592 floats per half-segment
HPP = SEGS * 2 // PPART  # 15 half-segments per partition
QCOL = HALF // 4  # 648 floats per quarter-column
NQ = HPP * 4  # 60 quarter-columns per partition

# Chunk sizes in quarter-columns (sum = 60), decreasing so the vector
# engine's reduce backlog drains before the final chunk lands.
CHUNKS = (19, 13, 9, 6, 4, 3, 2, 1, 1, 1, 1)

_NC_CACHE = {}


def _build_nc(chunks=CHUNKS):
    import concourse.bass as bass
    from concourse import mybir

    assert sum(chunks) == NQ

    f32 = mybir.dt.float32
    bf16 = mybir.dt.bfloat16
    X = mybir.AxisListType.X
    MULT = mybir.AluOpType.mult
    MAX = mybir.AluOpType.max

    nc = bass.Bass("TRN2")

    # The constructor registers four const-APs via gpsimd.memset; nothing in
    # this kernel reads them (const_aps are only consumed by
    # scalar.activation bias handling), but MEMSET counts as "useful" to the
    # profiler, so they start the measured window ~1.4us before the first
    # load issue. Strip them.
    memset_names = set()
    for name, inst in list(nc.inst_map.items()):
        if isinstance(inst, mybir.InstMemset):
            assert inst.sync_info is None or not inst.sync_info.on_update
            memset_names.add(name)
            del nc.inst_map[name]
    for f in nc.m.functions:
        for blk in f.blocks:
            blk.instructions = [
                i for i in blk.instructions if i.name not in memset_names
            ]

    ptm = nc.dram_tensor("ptm", [PPART, NQ, QCOL], f32, kind="ExternalInput")
    maskf = nc.dram_tensor("maskf", [ROWS, NMEM], f32, kind="ExternalInput")
    out = nc.dram_tensor("out", [ROWS], f32, kind="ExternalOutput")

    with (
        nc.Block() as block,
        nc.semaphore("ssem") as ssem,  # SWDGE load completions
        nc.semaphore("asem") as asem,  # scalar-ring DMA completions
        nc.semaphore("vsem") as vsem,  # vector's stats done
        nc.semaphore("csem") as csem,  # final result in SBUF
        nc.semaphore("tsem") as tsem,  # same-engine RAW serialization ticks
        nc.semaphore("osem") as osem,  # out DMA completion (walrus requires a sem)
        nc.sbuf_tensor("data", [PPART, NQ, QCOL], bf16) as data,
        nc.sbuf_tensor("qstats", [PPART, NQ], bf16) as qstats,
        nc.sbuf_tensor("stats", [PPART, HPP], bf16) as stats,
        nc.sbuf_tensor("maskt", [ROWS, NMEM], f32) as maskt,
        nc.sbuf_tensor("cnt", [ROWS, 1], f32) as cnt,
        nc.sbuf_tensor("rcnt", [ROWS, 1], f32) as rcnt,
        nc.sbuf_tensor("mx2", [ROWS, 2 * NMEM], bf16) as mx2,
        nc.sbuf_tensor("mx", [ROWS, NMEM], f32) as mx,
        nc.sbuf_tensor("prod", [ROWS, NMEM], f32) as prod,
        nc.sbuf_tensor("res", [ROWS, 1], f32) as res,
    ):
        bounds = []
        a = 0
        for w in chunks:
            bounds.append((a, a + w))
            a += w

        @block.gpsimd
        def _(gpsimd):
            # SWDGE casts f32->bf16 inline during the load: HBM reads are
            # unchanged (still the roofline) but SBUF writes halve and the
            # DVE reduce gets packed-bf16 2x throughput, so vector stays off
            # the critical path even when HBM delivers at >400 GB/s or the
            # DVE runs in its degraded ~820ns/qcol regime. bf16's <=0.4%
            # relative error is far inside the 2e-2 tolerance.
            for a, b in bounds:
                gpsimd.dma_start(data[:, a:b, :], ptm[:, a:b, :]).then_inc(ssem, 16)

        # repartition target view: mx2[r, 15*t + j] == stats[4r + t, j]
        # (both sides of each DMA walk half-segments in ascending order, so
        # mx2[r, 2*m+h] == max of half h of segment r*30+m).
        mx2g = mx2[:].rearrange("r (t j) -> r t j", j=HPP)

        @block.scalar
        def _(scalar):
            scalar.dma_start(maskt[:], maskf[:]).then_inc(asem, 16)
            # repartition A: stats cols 0..13 — issued while the last column
            # is still streaming, so its ~2us DMA cost hides under the loads.
            scalar.wait_ge(vsem, 1)
            scalar.dma_start(mx2g[:, :, 0 : HPP - 1], stats[:, 0 : HPP - 1]).then_inc(
                asem, 16
            )
            # repartition B: only col 14 (512 B) stays on the critical path.
            # 128x4B scattered descriptors — tiny, and latency-bound anyway.
            scalar.wait_ge(vsem, 2)
            with nc.allow_non_contiguous_dma("128 4B descs, 512B total"):
                scalar.dma_start(
                    mx2g[:, :, HPP - 1 : HPP], stats[:, HPP - 1 : HPP]
                ).then_inc(asem, 16)

        @block.vector
        def _(vector):
            # chunk index after which qcols 0..55 (= stats cols 0..13) are done
            splitk = [i for i, (_, b) in enumerate(bounds) if b == 4 * (HPP - 1)]
            assert len(splitk) == 1, "need a chunk boundary at the last column"
            splitk = splitk[0]
            qv = qstats[:].rearrange("p (h q) -> p h q", q=4)

            vector.wait_ge(asem, 16)
            vector.reduce_sum(out=cnt[:], in_=maskt[:], axis=X).then_inc(tsem, 1)
            vector.wait_ge(tsem, 1)
            vector.reciprocal(out=rcnt[:], in_=cnt[:])
            for k, (a, b) in enumerate(bounds):
                vector.wait_ge(ssem, 16 * (k + 1))
                vector.reduce_max(
                    out=qstats[:, a:b], in_=data[:, a:b, :], axis=X
                ).then_inc(tsem, 1)
                if k == splitk:
                    # join A: fold quarters of cols 0..13 while col 14 streams
                    vector.wait_ge(tsem, 2 + k)
                    vector.reduce_max(
                        out=stats[:, 0 : HPP - 1], in_=qv[:, 0 : HPP - 1, :], axis=X
                    ).then_inc(vsem, 1)
            vector.wait_ge(tsem, 1 + len(bounds))
            # join B: fold col 14's quarters -> triggers the tiny repart B
            vector.reduce_max(
                out=stats[:, HPP - 1 : HPP], in_=qv[:, HPP - 1 : HPP, :], axis=X
            ).then_inc(vsem, 1)
            vector.wait_ge(asem, 48)
            mx2v = mx2[:].rearrange("r (m two) -> r m two", two=2)
            # mx = max(half0, half1)
            vector.scalar_tensor_tensor(
                out=mx[:], in0=mx2v[:, :, 0], scalar=1.0, in1=mx2v[:, :, 1],
                op0=MULT, op1=MAX,
            ).then_inc(tsem, 1)
            vector.wait_ge(tsem, 2 + len(bounds))
            # prod = (mx * rcnt) * mask; res = row-sum(prod) = the output
            vector.scalar_tensor_tensor(
                out=prod[:], in0=mx[:], scalar=rcnt[:], in1=maskt[:],
                op0=MULT, op1=MULT, accum_out=res[:],
            ).then_inc(csem, 1)

        @block.sync
        def _(sync):
            sync.wait_ge(csem, 1)
            sync.dma_start(out[:], res[:, 0]).then_inc(osem, 16)

    return nc


def _get_nc():
    if "nc" not in _NC_CACHE:
        _NC_CACHE["nc"] = _build_nc()
    return _NC_CACHE["nc"]


def make_in_maps(ptm, mem_mask):
    ptm = np.ascontiguousarray(np.asarray(ptm, dtype=np.float32))
    mask = np.asarray(mem_mask)
    maskf = np.ascontiguousarray(mask.reshape(N, NMEM).astype(np.float32))
    ptm_flat = ptm.reshape(N * NMEM, FEAT)

    in_maps = []
    for i in range(NCORES):
        shard = ptm_flat[i * SEGS : (i + 1) * SEGS].reshape(PPART, NQ, QCOL)
        in_maps.append(
            {"ptm": shard, "maskf": maskf[i * ROWS : (i + 1) * ROWS]}
        )
    return in_maps


def _ensure_ntff_hook():
    """Register the axon NTFF profiling hook (the container's antenv lacks
    axon_hooks; synthesize it from trn_agent_boot), and stub the artifact
    upload which has no bucket access here."""
    import types

    try:
        from antenv.axon_hooks import get_axon_ntff_profile_hook  # noqa: F401
    except ImportError:
        import antenv
        from trn_agent_boot.trn_boot import _ntff_profile_via_ctypes

        mod = types.ModuleType("antenv.axon_hooks")
        mod._hook = _ntff_profile_via_ctypes("/opt/axon/libaxon_pjrt.so")
        mod.set_axon_ntff_profile_hook = lambda h: setattr(mod, "_hook", h)
        mod.get_axon_ntff_profile_hook = lambda: mod._hook
        sys.modules["antenv.axon_hooks"] = mod
        antenv.axon_hooks = mod

    from concourse import bass_utils

    if not getattr(bass_utils.upload_artifacts, "_stubbed", False):
        def _no_upload(tmpdir):
            return str(tmpdir)

        _no_upload._stubbed = True
        bass_utils.upload_artifacts = _no_upload


def run(ptm, mem_mask, trace=False):
    from concourse.bass_utils import run_bass_kernel_spmd

    if trace:
        _ensure_ntff_hook()

    in_maps = make_in_maps(ptm, mem_mask)

    nc = _get_nc()
    kr = run_bass_kernel_spmd(nc, in_maps, list(range(NCORES)), trace=trace)
    out = np.concatenate([np.asarray(kr.results[i]["out"]) for i in range(NCORES)])
    return out.astype(np.float32), kr


def kernel(ptm, mem_mask):
    out, _ = run(ptm, mem_mask, trace=False)
    return out


# revision 32
# speedup vs baseline: 1.0139x; 1.0139x over previous
"""Trainium2 Bass kernel for BinOverlapPredictionFromMaxProj (segment max + masked mean).

Full computation:
  ptm: (32, 8, 30, 1, 72, 72) f32, mem_mask: (32, 8, 30) bool
  n = 32*8 = 256 rows; per row: max over 5184-feature axis per mem (30), then
  masked mean over mems -> out (256,) f32.

Sharding: data-parallel over the 256 fused rows across 8 cores (32 rows each).
Per core: 960 segments x 5184 features (~19.9 MB) -> memory-bound.

Device plan per core (raw bass, no TileContext): the shard's flat stream is
viewed as (128 partitions, 60, 648) -- each partition row holds 15 aligned
half-segments of 2592 floats = 60 QUARTER-columns of 648 floats.
  - Streaming loads on the gpsimd SWDGE queue. SWDGE's completion semaphore
    is WAW-ordered behind the data writes (HWDGE's then_inc was measured
    firing up to ~10us before the bytes actually landed in SBUF -- unusable
    as a data-ready signal mid-stream -- and concurrent HWDGE writes also
    slowed DVE reduces ~20%).
  - Chunks geometrically DECREASE (19,13,9,6,4,3,2,1,1,1,1 qcols): DVE
    reduce costs ~0.69x the DMA time per byte, so sizes shrinking by >=~0.7
    let the vector engine drain its backlog and finish the last chunk's
    reduce right after the last DMA byte lands.
  - All reduce_max on vector (only DVE has free-axis reduce) into
    qstats (128, 60); one cheap join-reduce folds to (128, 15).
  - One small SBUF->SBUF DMA repartitions the 1920 half-maxes (128, 15) ->
    (32, 60) (both APs walk half-segments in ascending order); then TWO
    fused scalar_tensor_tensor ops finish: pairwise max of the half pairs,
    then (mx * rcnt) * mask with accum_out giving the row sum directly.
  - The DVE pipeline has no same-engine RAW interlock (a dependent op can
    sample inputs before the previous op's write commits), so dependent
    same-engine pairs are separated by tsem ticks.

Raw bass instead of TileContext: Tile's drain/barrier/sem-free teardown adds
~2-3us extra inside the measured window, and Tile's scheduler is unneeded
for this fixed pipeline. The NEFF wrapper's own epilogue (per-sem zeroing
spam, ~8us) is emitted either way; nothing in-kernel can remove it.
"""

import sys

import numpy as np

if "/opt/trn_rl_repo" not in sys.path:
    sys.path.insert(0, "/opt/trn_rl_repo")

NCORES = 8
NF, NS, NMEM, FEAT = 32, 8, 30, 5184
N = NF * NS  # 256
ROWS = N // NCORES  # 32 rows per core
SEGS = ROWS * NMEM  # 960 segments per core
PPART = 128  # partitions
HALF = FEAT // 2  # 2592 floats per half-segment
HPP = SEGS * 2 // PPART  # 15 half-segments per partition
QCOL = HALF // 4  # 648 floats per quarter-column
NQ = HPP * 4  # 60 quarter-columns per partition

# Chunk sizes in quarter-columns (sum = 60), decreasing so the vector
# engine's reduce backlog drains before the final chunk lands.
CHUNKS = (19, 13, 9, 6, 4, 3, 2, 1, 1, 1, 1)

_NC_CACHE = {}


def _build_nc(chunks=CHUNKS):
    import concourse.bass as bass
    from concourse import mybir

    assert sum(chunks) == NQ

    f32 = mybir.dt.float32
    bf16 = mybir.dt.bfloat16
    X = mybir.AxisListType.X
    MULT = mybir.AluOpType.mult
    MAX = mybir.AluOpType.max

    nc = bass.Bass("TRN2")

    # The constructor registers four const-APs via gpsimd.memset; nothing in
    # this kernel reads them (const_aps are only consumed by
    # scalar.activation bias handling), but MEMSET counts as "useful" to the
    # profiler, so they start the measured window ~1.4us before the first
    # load issue. Strip them.
    memset_names = set()
    for name, inst in list(nc.inst_map.items()):
        if isinstance(inst, mybir.InstMemset):
            assert inst.sync_info is None or not inst.sync_info.on_update
            memset_names.add(name)
            del nc.inst_map[name]
    for f in nc.m.functions:
        for blk in f.blocks:
            blk.instructions = [
                i for i in blk.instructions if i.name not in memset_names
            ]

    ptm = nc.dram_tensor("ptm", [PPART, NQ, QCOL], f32, kind="ExternalInput")
    maskf = nc.dram_tensor("maskf", [ROWS, NMEM], f32, kind="ExternalInput")
    out = nc.dram_tensor("out", [ROWS], f32, kind="ExternalOutput")

    with (
        nc.Block() as block,
        nc.semaphore("ssem") as ssem,  # SWDGE load completions
        nc.semaphore("asem") as asem,  # scalar-ring DMA completions
        nc.semaphore("vsem") as vsem,  # vector's stats done
        nc.semaphore("csem") as csem,  # final result in SBUF
        nc.semaphore("tsem") as tsem,  # same-engine RAW serialization ticks
        nc.semaphore("osem") as osem,  # out DMA completion (walrus requires a sem)
        nc.sbuf_tensor("data", [PPART, NQ, QCOL], bf16) as data,
        nc.sbuf_tensor("data32", [PPART, 1, QCOL], f32) as data32,
        nc.sbuf_tensor("fdata", [PPART, NQ, QCOL // 2], bf16) as fdata,
        nc.sbuf_tensor("qstats", [PPART, NQ], bf16) as qstats,
        nc.sbuf_tensor("stats", [PPART, HPP], bf16) as stats,
        nc.sbuf_tensor("maskt", [ROWS, NMEM], f32) as maskt,
        nc.sbuf_tensor("cnt", [ROWS, 1], f32) as cnt,
        nc.sbuf_tensor("rcnt", [ROWS, 1], f32) as rcnt,
        nc.sbuf_tensor("mx2", [ROWS, 2 * NMEM], bf16) as mx2,
        nc.sbuf_tensor("mx", [ROWS, NMEM], f32) as mx,
        nc.sbuf_tensor("prod", [ROWS, NMEM], f32) as prod,
        nc.sbuf_tensor("res", [ROWS, 1], f32) as res,
    ):
        bounds = []
        a = 0
        for w in chunks:
            bounds.append((a, a + w))
            a += w

        # repartition target view: mx2[r, 15*t + j] == stats[4r + t, j]
        # (both sides of each DMA walk half-segments in ascending order, so
        # mx2[r, 2*m+h] == max of half h of segment r*30+m).
        mx2g = mx2[:].rearrange("r (t j) -> r t j", j=HPP)

        @block.gpsimd
        def _(gpsimd):
            # SWDGE casts f32->bf16 inline during the load: HBM reads are
            # unchanged (still the roofline) but SBUF writes halve and the
            # DVE fold gets packed-bf16 2x throughput, so vector stays off
            # the critical path even when HBM delivers at >400 GB/s or the
            # DVE runs in its degraded ~820ns/qcol regime. bf16's <=0.4%
            # relative error is far inside the 2e-2 tolerance.
            # The LAST chunk is a plain f32 copy: the consumer side trusts a
            # chunk's data only once the NEXT chunk's (FIFO-later) semaphore
            # fires, and a plain copy's WAW-ordered inc anchors the tail.
            for a, b in bounds[:-1]:
                gpsimd.dma_start(data[:, a:b, :], ptm[:, a:b, :]).then_inc(ssem, 16)
            la, lb = bounds[-1]
            gpsimd.dma_start(data32[:], ptm[:, la:lb, :]).then_inc(ssem, 16)
            # Repartitions stay on SWDGE: its completion inc is WAW-ordered
            # behind the data writes. (On the HWDGE ring the inc was observed
            # firing before the bytes landed, which intermittently fed the
            # final combine uninitialized SBUF.)
            # A: stats cols 0..13, issued while col 14 still streams.
            gpsimd.wait_ge(vsem, 1)
            gpsimd.dma_start(mx2g[:, :, 0 : HPP - 1], stats[:, 0 : HPP - 1]).then_inc(
                ssem, 16
            )
            # B: only col 14 (128 scattered 2B descs) on the critical path.
            gpsimd.wait_ge(vsem, 2)
            with nc.allow_non_contiguous_dma("128 2B descs, 256B total"):
                gpsimd.dma_start(
                    mx2g[:, :, HPP - 1 : HPP], stats[:, HPP - 1 : HPP]
                ).then_inc(ssem, 16)

        @block.scalar
        def _(scalar):
            scalar.dma_start(maskt[:], maskf[:]).then_inc(asem, 16)

        @block.vector
        def _(vector):
            # chunk index after which qcols 0..55 (= stats cols 0..13) are done
            splitk = [i for i, (_, b) in enumerate(bounds) if b == 4 * (HPP - 1)]
            assert len(splitk) == 1, "need a chunk boundary at the last column"
            splitk = splitk[0]
            qv = qstats[:].rearrange("p (h q) -> p h q", q=4)

            # tsem tick counter: every dependent same-engine pair is split by
            # a producer .then_inc(tsem) + consumer wait (DVE has no RAW
            # interlock). `tick()` returns the wait threshold for the most
            # recent producer.
            t = [0]

            def tick(inst):
                inst.then_inc(tsem, 1)
                t[0] += 1
                return t[0]

            # The mask DMA completion rides the HWDGE ring, whose then_inc was
            # measured firing before the data lands. Gate the mask prep on
            # ssem>=16 too: chunk 0 takes ~16us of SWDGE streaming, a huge
            # margin for the 4KB mask, and vector has slack there anyway.
            vector.wait_ge(asem, 16)
            vector.wait_ge(ssem, 16)
            vector.wait_ge(
                tsem, tick(vector.reduce_sum(out=cnt[:], in_=maskt[:], axis=X))
            )
            vector.reciprocal(out=rcnt[:], in_=cnt[:])
            for k, (a, b) in enumerate(bounds[:-1]):
                # Trust chunk k's casted bytes only once chunk k+1's
                # (FIFO-later) semaphore fires: the cast path's completion
                # inc is not provably WAW-ordered behind the CCE-casted
                # writes, and a full chunk of queue lag (>=1us) covers any
                # write-pipeline depth.
                vector.wait_ge(ssem, 16 * (k + 2))
                # packed-bf16 tensor_tensor folds 648 -> 324 at 2 out/cycle
                # (4 inputs/cycle); the reduce then runs on half the data.
                # Together ~0.5us/qcol vs 0.82us/qcol for a straight reduce.
                vector.wait_ge(
                    tsem,
                    tick(
                        vector.tensor_max(
                            out=fdata[:, a:b, :],
                            in0=data[:, a:b, 0 : QCOL // 2],
                            in1=data[:, a:b, QCOL // 2 : QCOL],
                        )
                    ),
                )
                vector.reduce_max(
                    out=qstats[:, a:b], in_=fdata[:, a:b, :], axis=X
                ).then_inc(tsem, 1)
                t[0] += 1
                if k == splitk:
                    # join A: fold quarters of cols 0..13 while col 14 streams
                    vector.wait_ge(tsem, t[0])
                    vector.reduce_max(
                        out=stats[:, 0 : HPP - 1], in_=qv[:, 0 : HPP - 1, :], axis=X
                    ).then_inc(vsem, 1)
            # last chunk: plain f32, its own WAW-ordered semaphore
            la, lb = bounds[-1]
            vector.wait_ge(ssem, 16 * len(bounds))
            vector.reduce_max(
                out=qstats[:, la:lb], in_=data32[:], axis=X
            ).then_inc(tsem, 1)
            t[0] += 1
            vector.wait_ge(tsem, t[0])
            # join B: fold col 14's quarters -> triggers the tiny repart B
            vector.reduce_max(
                out=stats[:, HPP - 1 : HPP], in_=qv[:, HPP - 1 : HPP, :], axis=X
            ).then_inc(vsem, 1)
            vector.wait_ge(ssem, 16 * (len(bounds) + 2))  # both reparts landed
            mx2v = mx2[:].rearrange("r (m two) -> r m two", two=2)
            # mx = max(half0, half1)
            vector.wait_ge(
                tsem,
                tick(
                    vector.scalar_tensor_tensor(
                        out=mx[:], in0=mx2v[:, :, 0], scalar=1.0, in1=mx2v[:, :, 1],
                        op0=MULT, op1=MAX,
                    )
                ),
            )
            # prod = (mx * rcnt) * mask; res = row-sum(prod) = the output
            vector.scalar_tensor_tensor(
                out=prod[:], in0=mx[:], scalar=rcnt[:], in1=maskt[:],
                op0=MULT, op1=MULT, accum_out=res[:],
            ).then_inc(csem, 1)

        @block.sync
        def _(sync):
            sync.wait_ge(csem, 1)
            sync.dma_start(out[:], res[:, 0]).then_inc(osem, 16)

    return nc


def _get_nc():
    if "nc" not in _NC_CACHE:
        _NC_CACHE["nc"] = _build_nc()
    return _NC_CACHE["nc"]


def make_in_maps(ptm, mem_mask):
    ptm = np.ascontiguousarray(np.asarray(ptm, dtype=np.float32))
    mask = np.asarray(mem_mask)
    maskf = np.ascontiguousarray(mask.reshape(N, NMEM).astype(np.float32))
    ptm_flat = ptm.reshape(N * NMEM, FEAT)

    in_maps = []
    for i in range(NCORES):
        shard = ptm_flat[i * SEGS : (i + 1) * SEGS].reshape(PPART, NQ, QCOL)
        in_maps.append(
            {"ptm": shard, "maskf": maskf[i * ROWS : (i + 1) * ROWS]}
        )
    return in_maps


def _ensure_ntff_hook():
    """Register the axon NTFF profiling hook (the container's antenv lacks
    axon_hooks; synthesize it from trn_agent_boot), and stub the artifact
    upload which has no bucket access here."""
    import types

    try:
        from antenv.axon_hooks import get_axon_ntff_profile_hook  # noqa: F401
    except ImportError:
        import antenv
        from trn_agent_boot.trn_boot import _ntff_profile_via_ctypes

        mod = types.ModuleType("antenv.axon_hooks")
        mod._hook = _ntff_profile_via_ctypes("/opt/axon/libaxon_pjrt.so")
        mod.set_axon_ntff_profile_hook = lambda h: setattr(mod, "_hook", h)
        mod.get_axon_ntff_profile_hook = lambda: mod._hook
        sys.modules["antenv.axon_hooks"] = mod
        antenv.axon_hooks = mod

    from concourse import bass_utils

    if not getattr(bass_utils.upload_artifacts, "_stubbed", False):
        def _no_upload(tmpdir):
            return str(tmpdir)

        _no_upload._stubbed = True
        bass_utils.upload_artifacts = _no_upload


def run(ptm, mem_mask, trace=False):
    from concourse.bass_utils import run_bass_kernel_spmd

    if trace:
        _ensure_ntff_hook()

    in_maps = make_in_maps(ptm, mem_mask)

    nc = _get_nc()
    kr = run_bass_kernel_spmd(nc, in_maps, list(range(NCORES)), trace=trace)
    out = np.concatenate([np.asarray(kr.results[i]["out"]) for i in range(NCORES)])
    return out.astype(np.float32), kr


def kernel(ptm, mem_mask):
    out, _ = run(ptm, mem_mask, trace=False)
    return out


# revision 34
# speedup vs baseline: 1.0296x; 1.0155x over previous
"""Trainium2 Bass kernel for BinOverlapPredictionFromMaxProj (segment max + masked mean).

Full computation:
  ptm: (32, 8, 30, 1, 72, 72) f32, mem_mask: (32, 8, 30) bool
  n = 32*8 = 256 rows; per row: max over 5184-feature axis per mem (30), then
  masked mean over mems -> out (256,) f32.

Sharding: data-parallel over the 256 fused rows across 8 cores (32 rows each).
Per core: 960 segments x 5184 features (~19.9 MB) -> memory-bound.

Device plan per core (raw bass, no TileContext): the shard's flat stream is
viewed as (128 partitions, 60, 648) -- each partition row holds 15 aligned
half-segments of 2592 floats = 60 QUARTER-columns of 648 floats.
  - Streaming loads on the gpsimd SWDGE queue, cast f32->bf16 inline by the
    DMA. HBM reads (the roofline, ~334 GB/s/core with all 8 cores pulling)
    are unchanged, but SBUF writes halve and the DVE gets packed-bf16
    2x throughput for the fold below. SWDGE rather than HWDGE because
    HWDGE's then_inc was measured firing up to ~10us before the bytes
    actually landed in SBUF (unusable as a data-ready signal mid-stream),
    and concurrent HWDGE writes also slowed DVE reduces ~20%.
  - Chunks geometrically DECREASE (19,13,9,6,4,3,2,1,1,1,1 qcols) so the
    vector engine drains its backlog and finishes right after the last DMA
    byte lands, in both the nominal (~0.68ns/elem) and degraded
    (~0.82ns/elem, clock-throttled) DVE regimes.
  - Per chunk the DVE runs a packed-bf16 tensor_max FOLD 648 -> 324
    (2 outs/cycle/lane = 4 inputs/cycle) then reduce_max on the half-size
    fdata: ~0.5us/qcol total vs 0.82us/qcol for a straight reduce, keeping
    vector off the critical path even at >400 GB/s HBM delivery.
  - The casting DMA's completion inc is NOT provably WAW-ordered behind its
    CCE-casted writes (intermittent inf/garbage when consumed immediately),
    so chunk k is consumed only after chunk k+1's FIFO-later semaphore
    fires; the last chunk is a plain f32 copy whose inc IS write-ordered,
    anchoring the tail.
  - quarter-maxes land in qstats (128, 60); join-reduces fold to stats
    (128, 15); two SBUF->SBUF SWDGE DMAs repartition to mx2 (32, 60) (both
    APs walk half-segments in ascending order): cols 0..13 early (hidden
    under the stream), the 512B col-14 piece on the critical path. Then TWO
    fused scalar_tensor_tensor ops finish: pairwise max of the half pairs,
    then (mx * rcnt) * mask with accum_out giving the row sum directly.
  - The DVE pipeline has no same-engine RAW interlock (a dependent op can
    sample inputs before the previous op's write commits; measured garbage
    reciprocal), so dependent same-engine pairs are separated by tsem ticks.

Raw bass instead of TileContext: Tile's drain/barrier/sem-free teardown adds
~2-3us extra inside the measured window, and Tile's scheduler is unneeded
for this fixed pipeline. The NEFF wrapper's own epilogue (per-sem zeroing
spam, ~8us) is emitted either way; nothing in-kernel can remove it.
"""

import sys

import numpy as np

if "/opt/trn_rl_repo" not in sys.path:
    sys.path.insert(0, "/opt/trn_rl_repo")

NCORES = 8
NF, NS, NMEM, FEAT = 32, 8, 30, 5184
N = NF * NS  # 256
ROWS = N // NCORES  # 32 rows per core
SEGS = ROWS * NMEM  # 960 segments per core
PPART = 128  # partitions
HALF = FEAT // 2  # 2592 floats per half-segment
HPP = SEGS * 2 // PPART  # 15 half-segments per partition
QCOL = HALF // 4  # 648 floats per quarter-column
NQ = HPP * 4  # 60 quarter-columns per partition

# Chunk sizes in quarter-columns (sum = 60), decreasing so the vector
# engine's reduce backlog drains before the final chunk lands.
CHUNKS = (19, 13, 9, 6, 4, 3, 2, 1, 1, 1, 1)

_NC_CACHE = {}


def _build_nc(chunks=CHUNKS):
    import concourse.bass as bass
    from concourse import mybir

    assert sum(chunks) == NQ
    assert chunks[-1] == 1, "last chunk is the plain-f32 tail anchor"

    f32 = mybir.dt.float32
    bf16 = mybir.dt.bfloat16
    X = mybir.AxisListType.X
    MULT = mybir.AluOpType.mult
    MAX = mybir.AluOpType.max

    nc = bass.Bass("TRN2")

    # The constructor registers four const-APs via gpsimd.memset; nothing in
    # this kernel reads them (const_aps are only consumed by
    # scalar.activation bias handling), but MEMSET counts as "useful" to the
    # profiler, so they start the measured window ~1.4us before the first
    # load issue. Strip them.
    memset_names = set()
    for name, inst in list(nc.inst_map.items()):
        if isinstance(inst, mybir.InstMemset):
            assert inst.sync_info is None or not inst.sync_info.on_update
            memset_names.add(name)
            del nc.inst_map[name]
    for f in nc.m.functions:
        for blk in f.blocks:
            blk.instructions = [
                i for i in blk.instructions if i.name not in memset_names
            ]

    ptm = nc.dram_tensor("ptm", [PPART, NQ, QCOL], f32, kind="ExternalInput")
    maskf = nc.dram_tensor("maskf", [ROWS, NMEM], f32, kind="ExternalInput")
    out = nc.dram_tensor("out", [ROWS], f32, kind="ExternalOutput")

    with (
        nc.Block() as block,
        nc.semaphore("ssem") as ssem,  # SWDGE load completions
        nc.semaphore("asem") as asem,  # scalar-ring DMA completions
        nc.semaphore("vsem") as vsem,  # vector's stats done
        nc.semaphore("csem") as csem,  # final result in SBUF
        nc.semaphore("tsem") as tsem,  # same-engine RAW serialization ticks
        nc.semaphore("osem") as osem,  # out DMA completion (walrus requires a sem)
        nc.sbuf_tensor("data", [PPART, NQ, QCOL], bf16) as data,
        nc.sbuf_tensor("data32", [PPART, 1, QCOL], f32) as data32,
        nc.sbuf_tensor("fdata", [PPART, NQ, QCOL // 2], bf16) as fdata,
        nc.sbuf_tensor("qstats", [PPART, NQ], bf16) as qstats,
        nc.sbuf_tensor("stats", [PPART, HPP], bf16) as stats,
        nc.sbuf_tensor("maskt", [ROWS, NMEM], f32) as maskt,
        nc.sbuf_tensor("cnt", [ROWS, 1], f32) as cnt,
        nc.sbuf_tensor("rcnt", [ROWS, 1], f32) as rcnt,
        nc.sbuf_tensor("mx2", [ROWS, 2 * NMEM], bf16) as mx2,
        nc.sbuf_tensor("mx", [ROWS, NMEM], f32) as mx,
        nc.sbuf_tensor("prod", [ROWS, NMEM], f32) as prod,
        nc.sbuf_tensor("res", [ROWS, 1], f32) as res,
    ):
        bounds = []
        a = 0
        for w in chunks:
            bounds.append((a, a + w))
            a += w

        # repartition target view: mx2[r, 15*t + j] == stats[4r + t, j]
        # (both sides of each DMA walk half-segments in ascending order, so
        # mx2[r, 2*m+h] == max of half h of segment r*30+m).
        mx2g = mx2[:].rearrange("r (t j) -> r t j", j=HPP)

        @block.gpsimd
        def _(gpsimd):
            # SWDGE casts f32->bf16 inline during the load: HBM reads are
            # unchanged (still the roofline) but SBUF writes halve and the
            # DVE fold gets packed-bf16 2x throughput, so vector stays off
            # the critical path even when HBM delivers at >400 GB/s or the
            # DVE runs in its degraded ~820ns/qcol regime. bf16's <=0.4%
            # relative error is far inside the 2e-2 tolerance.
            # The LAST chunk is a plain f32 copy: the consumer side trusts a
            # chunk's data only once the NEXT chunk's (FIFO-later) semaphore
            # fires, and a plain copy's WAW-ordered inc anchors the tail.
            for a, b in bounds[:-1]:
                gpsimd.dma_start(data[:, a:b, :], ptm[:, a:b, :]).then_inc(ssem, 16)
            la, lb = bounds[-1]
            gpsimd.dma_start(data32[:], ptm[:, la:lb, :]).then_inc(ssem, 16)
            # Repartitions stay on SWDGE: its completion inc is WAW-ordered
            # behind the data writes. (On the HWDGE ring the inc was observed
            # firing before the bytes landed, which intermittently fed the
            # final combine uninitialized SBUF.)
            # A: stats cols 0..13, issued while col 14 still streams.
            gpsimd.wait_ge(vsem, 1)
            gpsimd.dma_start(mx2g[:, :, 0 : HPP - 1], stats[:, 0 : HPP - 1]).then_inc(
                ssem, 16
            )
            # B: only col 14 (128 scattered 2B descs) on the critical path.
            gpsimd.wait_ge(vsem, 2)
            with nc.allow_non_contiguous_dma("128 2B descs, 256B total"):
                gpsimd.dma_start(
                    mx2g[:, :, HPP - 1 : HPP], stats[:, HPP - 1 : HPP]
                ).then_inc(ssem, 16)

        @block.scalar
        def _(scalar):
            scalar.dma_start(maskt[:], maskf[:]).then_inc(asem, 16)

        @block.vector
        def _(vector):
            # chunk index after which qcols 0..55 (= stats cols 0..13) are done
            splitk = [i for i, (_, b) in enumerate(bounds) if b == 4 * (HPP - 1)]
            assert len(splitk) == 1, "need a chunk boundary at the last column"
            splitk = splitk[0]
            qv = qstats[:].rearrange("p (h q) -> p h q", q=4)

            # tsem tick counter: every dependent same-engine pair is split by
            # a producer .then_inc(tsem) + consumer wait (DVE has no RAW
            # interlock). `tick()` returns the wait threshold for the most
            # recent producer.
            t = [0]

            def tick(inst):
                inst.then_inc(tsem, 1)
                t[0] += 1
                return t[0]

            # The mask DMA completion rides the HWDGE ring, whose then_inc was
            # measured firing before the data lands. Gate the mask prep on
            # ssem>=16 too: chunk 0 takes ~16us of SWDGE streaming, a huge
            # margin for the 4KB mask, and vector has slack there anyway.
            vector.wait_ge(asem, 16)
            vector.wait_ge(ssem, 16)
            vector.wait_ge(
                tsem, tick(vector.reduce_sum(out=cnt[:], in_=maskt[:], axis=X))
            )
            vector.reciprocal(out=rcnt[:], in_=cnt[:])
            for k, (a, b) in enumerate(bounds[:-1]):
                # Trust chunk k's casted bytes only once chunk k+1's
                # (FIFO-later) semaphore fires: the cast path's completion
                # inc is not provably WAW-ordered behind the CCE-casted
                # writes, and a full chunk of queue lag (>=1us) covers any
                # write-pipeline depth.
                vector.wait_ge(ssem, 16 * (k + 2))
                # packed-bf16 tensor_tensor folds 648 -> 324 at 2 out/cycle
                # (4 inputs/cycle); the reduce then runs on half the data.
                # Together ~0.5us/qcol vs 0.82us/qcol for a straight reduce.
                vector.wait_ge(
                    tsem,
                    tick(
                        vector.tensor_max(
                            out=fdata[:, a:b, :],
                            in0=data[:, a:b, 0 : QCOL // 2],
                            in1=data[:, a:b, QCOL // 2 : QCOL],
                        )
                    ),
                )
                vector.reduce_max(
                    out=qstats[:, a:b], in_=fdata[:, a:b, :], axis=X
                ).then_inc(tsem, 1)
                t[0] += 1
                if k == splitk:
                    # join A: fold quarters of cols 0..13 while col 14 streams
                    vector.wait_ge(tsem, t[0])
                    vector.reduce_max(
                        out=stats[:, 0 : HPP - 1], in_=qv[:, 0 : HPP - 1, :], axis=X
                    ).then_inc(vsem, 1)
            # last chunk: plain f32, its own WAW-ordered semaphore
            la, lb = bounds[-1]
            vector.wait_ge(ssem, 16 * len(bounds))
            vector.reduce_max(
                out=qstats[:, la:lb], in_=data32[:], axis=X
            ).then_inc(tsem, 1)
            t[0] += 1
            vector.wait_ge(tsem, t[0])
            # join B: fold col 14's quarters -> triggers the tiny repart B
            vector.reduce_max(
                out=stats[:, HPP - 1 : HPP], in_=qv[:, HPP - 1 : HPP, :], axis=X
            ).then_inc(vsem, 1)
            vector.wait_ge(ssem, 16 * (len(bounds) + 2))  # both reparts landed
            mx2v = mx2[:].rearrange("r (m two) -> r m two", two=2)
            # mx = max(half0, half1)
            vector.wait_ge(
                tsem,
                tick(
                    vector.scalar_tensor_tensor(
                        out=mx[:], in0=mx2v[:, :, 0], scalar=1.0, in1=mx2v[:, :, 1],
                        op0=MULT, op1=MAX,
                    )
                ),
            )
            # prod = (mx * rcnt) * mask; res = row-sum(prod) = the output
            vector.scalar_tensor_tensor(
                out=prod[:], in0=mx[:], scalar=rcnt[:], in1=maskt[:],
                op0=MULT, op1=MULT, accum_out=res[:],
            ).then_inc(csem, 1)

        @block.sync
        def _(sync):
            sync.wait_ge(csem, 1)
            sync.dma_start(out[:], res[:, 0]).then_inc(osem, 16)

    return nc


def _get_nc():
    if "nc" not in _NC_CACHE:
        _NC_CACHE["nc"] = _build_nc()
    return _NC_CACHE["nc"]


def make_in_maps(ptm, mem_mask):
    ptm = np.ascontiguousarray(np.asarray(ptm, dtype=np.float32))
    mask = np.asarray(mem_mask)
    maskf = np.ascontiguousarray(mask.reshape(N, NMEM).astype(np.float32))
    ptm_flat = ptm.reshape(N * NMEM, FEAT)

    in_maps = []
    for i in range(NCORES):
        shard = ptm_flat[i * SEGS : (i + 1) * SEGS].reshape(PPART, NQ, QCOL)
        in_maps.append(
            {"ptm": shard, "maskf": maskf[i * ROWS : (i + 1) * ROWS]}
        )
    return in_maps


def _ensure_ntff_hook():
    """Register the axon NTFF profiling hook (the container's antenv lacks
    axon_hooks; synthesize it from trn_agent_boot), and stub the artifact
    upload which has no bucket access here."""
    import types

    try:
        from antenv.axon_hooks import get_axon_ntff_profile_hook  # noqa: F401
    except ImportError:
        import antenv
        from trn_agent_boot.trn_boot import _ntff_profile_via_ctypes

        mod = types.ModuleType("antenv.axon_hooks")
        mod._hook = _ntff_profile_via_ctypes("/opt/axon/libaxon_pjrt.so")
        mod.set_axon_ntff_profile_hook = lambda h: setattr(mod, "_hook", h)
        mod.get_axon_ntff_profile_hook = lambda: mod._hook
        sys.modules["antenv.axon_hooks"] = mod
        antenv.axon_hooks = mod

    from concourse import bass_utils

    if not getattr(bass_utils.upload_artifacts, "_stubbed", False):
        def _no_upload(tmpdir):
            return str(tmpdir)

        _no_upload._stubbed = True
        bass_utils.upload_artifacts = _no_upload


def run(ptm, mem_mask, trace=False):
    from concourse.bass_utils import run_bass_kernel_spmd

    if trace:
        _ensure_ntff_hook()

    in_maps = make_in_maps(ptm, mem_mask)

    nc = _get_nc()
    kr = run_bass_kernel_spmd(nc, in_maps, list(range(NCORES)), trace=trace)
    out = np.concatenate([np.asarray(kr.results[i]["out"]) for i in range(NCORES)])
    return out.astype(np.float32), kr


def kernel(ptm, mem_mask):
    out, _ = run(ptm, mem_mask, trace=False)
    return out


# revision 39
# speedup vs baseline: 1.1009x; 1.0693x over previous
"""Trainium2 Bass kernel for BinOverlapPredictionFromMaxProj (segment max + masked mean).

Full computation:
  ptm: (32, 8, 30, 1, 72, 72) f32, mem_mask: (32, 8, 30) bool
  n = 32*8 = 256 rows; per row: max over 5184-feature axis per mem (30), then
  masked mean over mems -> out (256,) f32.

Sharding: data-parallel over the 256 fused rows across 8 cores (32 rows each).
Per core: 960 segments x 5184 features (~19.9 MB) -> memory-bound.

Device plan per core (raw bass, no TileContext): the shard's flat stream is
viewed as (128 partitions, 60, 648) -- each partition row holds 15 aligned
half-segments of 2592 floats = 60 QUARTER-columns of 648 floats.
  - Streaming loads on the gpsimd SWDGE queue, cast f32->bf16 inline by the
    DMA. HBM reads (the roofline, ~334 GB/s/core with all 8 cores pulling)
    are unchanged, but SBUF writes halve and the DVE gets packed-bf16
    2x throughput for the fold below. SWDGE rather than HWDGE because
    HWDGE's then_inc was measured firing up to ~10us before the bytes
    actually landed in SBUF (unusable as a data-ready signal mid-stream),
    and concurrent HWDGE writes also slowed DVE reduces ~20%.
  - Chunks geometrically DECREASE (19,13,9,6,4,3,2,1,1,1,1 qcols) so the
    vector engine drains its backlog and finishes right after the last DMA
    byte lands, in both the nominal (~0.68ns/elem) and degraded
    (~0.82ns/elem, clock-throttled) DVE regimes.
  - Per chunk the DVE runs a packed-bf16 tensor_max FOLD 648 -> 324
    (2 outs/cycle/lane = 4 inputs/cycle) then reduce_max on the half-size
    fdata: ~0.5us/qcol total vs 0.82us/qcol for a straight reduce, keeping
    vector off the critical path even at >400 GB/s HBM delivery.
  - The casting DMA's completion inc is NOT provably WAW-ordered behind its
    CCE-casted writes (intermittent inf/garbage when consumed immediately),
    so chunk k is consumed only after chunk k+1's FIFO-later semaphore
    fires; the last chunk is a plain f32 copy whose inc IS write-ordered,
    anchoring the tail.
  - quarter-maxes land in qstats (128, 60); join-reduces fold to stats
    (128, 15); two SBUF->SBUF SWDGE DMAs repartition to mx2 (32, 60) (both
    APs walk half-segments in ascending order): cols 0..13 early (hidden
    under the stream), the 512B col-14 piece on the critical path. Then TWO
    fused scalar_tensor_tensor ops finish: pairwise max of the half pairs,
    then (mx * rcnt) * mask with accum_out giving the row sum directly.
  - The DVE pipeline has no same-engine RAW interlock (a dependent op can
    sample inputs before the previous op's write commits; measured garbage
    reciprocal), so dependent same-engine pairs are separated by tsem ticks.

Raw bass instead of TileContext: Tile's drain/barrier/sem-free teardown adds
~2-3us extra inside the measured window, and Tile's scheduler is unneeded
for this fixed pipeline. The NEFF wrapper's own epilogue (per-sem zeroing
spam, ~8us) is emitted either way; nothing in-kernel can remove it.
"""

import sys

import numpy as np

if "/opt/trn_rl_repo" not in sys.path:
    sys.path.insert(0, "/opt/trn_rl_repo")

NCORES = 8
NF, NS, NMEM, FEAT = 32, 8, 30, 5184
N = NF * NS  # 256
ROWS = N // NCORES  # 32 rows per core
SEGS = ROWS * NMEM  # 960 segments per core
PPART = 128  # partitions
HALF = FEAT // 2  # 2592 floats per half-segment
HPP = SEGS * 2 // PPART  # 15 half-segments per partition
QCOL = HALF // 4  # 648 floats per quarter-column
NQ = HPP * 4  # 60 quarter-columns per partition

# Chunk sizes in quarter-columns (sum = 60), decreasing so the vector
# engine's reduce backlog drains before the final chunk lands.
CHUNKS = (19, 13, 9, 6, 4, 3, 2, 1, 1, 1, 1)

_NC_CACHE = {}


def _build_nc(chunks=CHUNKS):
    import concourse.bass as bass
    from concourse import mybir

    assert sum(chunks) == NQ
    assert chunks[-1] == 1 and chunks[-2] == 1, "last two chunks anchor the f32 tail"

    f32 = mybir.dt.float32
    bf16 = mybir.dt.bfloat16
    X = mybir.AxisListType.X
    MULT = mybir.AluOpType.mult
    MAX = mybir.AluOpType.max

    nc = bass.Bass("TRN2")

    # The constructor registers four const-APs via gpsimd.memset; nothing in
    # this kernel reads them (const_aps are only consumed by
    # scalar.activation bias handling), but MEMSET counts as "useful" to the
    # profiler, so they start the measured window ~1.4us before the first
    # load issue. Strip them.
    memset_names = set()
    for name, inst in list(nc.inst_map.items()):
        if isinstance(inst, mybir.InstMemset):
            assert inst.sync_info is None or not inst.sync_info.on_update
            memset_names.add(name)
            del nc.inst_map[name]
    for f in nc.m.functions:
        for blk in f.blocks:
            blk.instructions = [
                i for i in blk.instructions if i.name not in memset_names
            ]

    ptm = nc.dram_tensor("ptm", [PPART, NQ, QCOL], f32, kind="ExternalInput")
    maskf = nc.dram_tensor("maskf", [ROWS, NMEM], f32, kind="ExternalInput")
    out = nc.dram_tensor("out", [ROWS], f32, kind="ExternalOutput")

    with (
        nc.Block() as block,
        nc.semaphore("ssem") as ssem,  # SWDGE load completions
        nc.semaphore("asem") as asem,  # scalar-ring DMA completions
        nc.semaphore("vsem") as vsem,  # vector's stats done
        nc.semaphore("csem") as csem,  # final result in SBUF
        nc.semaphore("tsem") as tsem,  # same-engine RAW serialization ticks
        nc.semaphore("osem") as osem,  # out DMA completion (walrus requires a sem)
        nc.sbuf_tensor("data", [PPART, NQ, QCOL], bf16) as data,
        nc.sbuf_tensor("data32", [PPART, 2, QCOL], f32) as data32,
        nc.sbuf_tensor("fdata", [PPART, NQ, QCOL // 2], bf16) as fdata,
        nc.sbuf_tensor("qstats", [PPART, NQ], bf16) as qstats,
        nc.sbuf_tensor("stats", [PPART, HPP], bf16) as stats,
        nc.sbuf_tensor("maskt", [ROWS, NMEM], f32) as maskt,
        nc.sbuf_tensor("cnt", [ROWS, 1], f32) as cnt,
        nc.sbuf_tensor("rcnt", [ROWS, 1], f32) as rcnt,
        nc.sbuf_tensor("mx2", [ROWS, 2 * NMEM], bf16) as mx2,
        nc.sbuf_tensor("mx", [ROWS, NMEM], f32) as mx,
        nc.sbuf_tensor("prod", [ROWS, NMEM], f32) as prod,
        nc.sbuf_tensor("res", [ROWS, 1], f32) as res,
    ):
        bounds = []
        a = 0
        for w in chunks:
            bounds.append((a, a + w))
            a += w

        # repartition target view: mx2[r, 15*t + j] == stats[4r + t, j]
        # (both sides of each DMA walk half-segments in ascending order, so
        # mx2[r, 2*m+h] == max of half h of segment r*30+m).
        mx2g = mx2[:].rearrange("r (t j) -> r t j", j=HPP)

        @block.gpsimd
        def _(gpsimd):
            # SWDGE casts f32->bf16 inline during the load: HBM reads are
            # unchanged (still the roofline) but SBUF writes halve and the
            # DVE fold gets packed-bf16 2x throughput, so vector stays off
            # the critical path even when HBM delivers at >400 GB/s or the
            # DVE runs in its degraded ~820ns/qcol regime. bf16's <=0.4%
            # relative error is far inside the 2e-2 tolerance.
            # The LAST TWO chunks are plain f32 copies: the consumer side
            # trusts a casted chunk's data only once the NEXT chunk's
            # (FIFO-later) semaphore fires, and a plain copy's WAW-ordered
            # inc anchors the tail (and lets chunk n-2's reduce overlap the
            # final chunk's streaming instead of serializing after it).
            for a, b in bounds[:-2]:
                gpsimd.dma_start(data[:, a:b, :], ptm[:, a:b, :]).then_inc(ssem, 16)
            for i, (a, b) in enumerate(bounds[-2:]):
                gpsimd.dma_start(
                    data32[:, i : i + 1, :], ptm[:, a:b, :]
                ).then_inc(ssem, 16)
            # Repartitions stay on SWDGE: its completion inc is WAW-ordered
            # behind the data writes. (On the HWDGE ring the inc was observed
            # firing before the bytes landed, which intermittently fed the
            # final combine uninitialized SBUF.)
            # A: stats cols 0..13, issued while col 14 still streams.
            gpsimd.wait_ge(vsem, 1)
            gpsimd.dma_start(mx2g[:, :, 0 : HPP - 1], stats[:, 0 : HPP - 1]).then_inc(
                ssem, 16
            )
            # B: only col 14 (128 scattered 2B descs) on the critical path.
            gpsimd.wait_ge(vsem, 2)
            with nc.allow_non_contiguous_dma("128 2B descs, 256B total"):
                gpsimd.dma_start(
                    mx2g[:, :, HPP - 1 : HPP], stats[:, HPP - 1 : HPP]
                ).then_inc(ssem, 16)

        @block.scalar
        def _(scalar):
            scalar.dma_start(maskt[:], maskf[:]).then_inc(asem, 16)

        @block.vector
        def _(vector):
            # chunk index after which qcols 0..55 (= stats cols 0..13) are done
            splitk = [i for i, (_, b) in enumerate(bounds) if b == 4 * (HPP - 1)]
            assert len(splitk) == 1, "need a chunk boundary at the last column"
            splitk = splitk[0]
            qv = qstats[:].rearrange("p (h q) -> p h q", q=4)

            # tsem tick counter: every dependent same-engine pair is split by
            # a producer .then_inc(tsem) + consumer wait (DVE has no RAW
            # interlock). `tick()` returns the wait threshold for the most
            # recent producer.
            t = [0]

            def tick(inst):
                inst.then_inc(tsem, 1)
                t[0] += 1
                return t[0]

            # The mask DMA completion rides the HWDGE ring, whose then_inc was
            # measured firing before the data lands. Gate the mask prep on
            # ssem>=16 too: chunk 0 takes ~16us of SWDGE streaming, a huge
            # margin for the 4KB mask, and vector has slack there anyway.
            vector.wait_ge(asem, 16)
            vector.wait_ge(ssem, 16)
            vector.wait_ge(
                tsem, tick(vector.reduce_sum(out=cnt[:], in_=maskt[:], axis=X))
            )
            vector.reciprocal(out=rcnt[:], in_=cnt[:])
            for k, (a, b) in enumerate(bounds[:-2]):
                # Trust chunk k's casted bytes only once chunk k+1's
                # (FIFO-later) semaphore fires: the cast path's completion
                # inc is not provably WAW-ordered behind the CCE-casted
                # writes, and a full chunk of queue lag (>=1us) covers any
                # write-pipeline depth.
                vector.wait_ge(ssem, 16 * (k + 2))
                # packed-bf16 tensor_tensor folds 648 -> 324 at 2 out/cycle
                # (4 inputs/cycle); the reduce then runs on half the data.
                # Together ~0.5us/qcol vs 0.82us/qcol for a straight reduce.
                vector.wait_ge(
                    tsem,
                    tick(
                        vector.tensor_max(
                            out=fdata[:, a:b, :],
                            in0=data[:, a:b, 0 : QCOL // 2],
                            in1=data[:, a:b, QCOL // 2 : QCOL],
                        )
                    ),
                )
                vector.reduce_max(
                    out=qstats[:, a:b], in_=fdata[:, a:b, :], axis=X
                ).then_inc(tsem, 1)
                t[0] += 1
                if k == splitk:
                    # join A: fold quarters of cols 0..13 while col 14 streams
                    vector.wait_ge(tsem, t[0])
                    vector.reduce_max(
                        out=stats[:, 0 : HPP - 1], in_=qv[:, 0 : HPP - 1, :], axis=X
                    ).then_inc(vsem, 1)
            # last two chunks: plain f32, each on its own WAW-ordered sem
            for i, (a, b) in enumerate(bounds[-2:]):
                vector.wait_ge(ssem, 16 * (len(bounds) - 1 + i))
                vector.reduce_max(
                    out=qstats[:, a:b], in_=data32[:, i : i + 1, :], axis=X
                ).then_inc(tsem, 1)
                t[0] += 1
            vector.wait_ge(tsem, t[0])
            # join B: fold col 14's quarters -> triggers the tiny repart B
            vector.reduce_max(
                out=stats[:, HPP - 1 : HPP], in_=qv[:, HPP - 1 : HPP, :], axis=X
            ).then_inc(vsem, 1)
            vector.wait_ge(ssem, 16 * (len(bounds) + 2))  # both reparts landed
            mx2v = mx2[:].rearrange("r (m two) -> r m two", two=2)
            # mx = max(half0, half1)
            vector.wait_ge(
                tsem,
                tick(
                    vector.scalar_tensor_tensor(
                        out=mx[:], in0=mx2v[:, :, 0], scalar=1.0, in1=mx2v[:, :, 1],
                        op0=MULT, op1=MAX,
                    )
                ),
            )
            # prod = (mx * rcnt) * mask; res = row-sum(prod) = the output
            vector.scalar_tensor_tensor(
                out=prod[:], in0=mx[:], scalar=rcnt[:], in1=maskt[:],
                op0=MULT, op1=MULT, accum_out=res[:],
            ).then_inc(csem, 1)

        @block.sync
        def _(sync):
            sync.wait_ge(csem, 1)
            sync.dma_start(out[:], res[:, 0]).then_inc(osem, 16)

    return nc


def _get_nc():
    if "nc" not in _NC_CACHE:
        _NC_CACHE["nc"] = _build_nc()
    return _NC_CACHE["nc"]


def make_in_maps(ptm, mem_mask):
    ptm = np.ascontiguousarray(np.asarray(ptm, dtype=np.float32))
    mask = np.asarray(mem_mask)
    maskf = np.ascontiguousarray(mask.reshape(N, NMEM).astype(np.float32))
    ptm_flat = ptm.reshape(N * NMEM, FEAT)

    in_maps = []
    for i in range(NCORES):
        shard = ptm_flat[i * SEGS : (i + 1) * SEGS].reshape(PPART, NQ, QCOL)
        in_maps.append(
            {"ptm": shard, "maskf": maskf[i * ROWS : (i + 1) * ROWS]}
        )
    return in_maps


def _ensure_ntff_hook():
    """Register the axon NTFF profiling hook (the container's antenv lacks
    axon_hooks; synthesize it from trn_agent_boot), and stub the artifact
    upload which has no bucket access here."""
    import types

    try:
        from antenv.axon_hooks import get_axon_ntff_profile_hook  # noqa: F401
    except ImportError:
        import antenv
        from trn_agent_boot.trn_boot import _ntff_profile_via_ctypes

        mod = types.ModuleType("antenv.axon_hooks")
        mod._hook = _ntff_profile_via_ctypes("/opt/axon/libaxon_pjrt.so")
        mod.set_axon_ntff_profile_hook = lambda h: setattr(mod, "_hook", h)
        mod.get_axon_ntff_profile_hook = lambda: mod._hook
        sys.modules["antenv.axon_hooks"] = mod
        antenv.axon_hooks = mod

    from concourse import bass_utils

    if not getattr(bass_utils.upload_artifacts, "_stubbed", False):
        def _no_upload(tmpdir):
            return str(tmpdir)

        _no_upload._stubbed = True
        bass_utils.upload_artifacts = _no_upload


def run(ptm, mem_mask, trace=False):
    from concourse.bass_utils import run_bass_kernel_spmd

    if trace:
        _ensure_ntff_hook()

    in_maps = make_in_maps(ptm, mem_mask)

    nc = _get_nc()
    kr = run_bass_kernel_spmd(nc, in_maps, list(range(NCORES)), trace=trace)
    out = np.concatenate([np.asarray(kr.results[i]["out"]) for i in range(NCORES)])
    return out.astype(np.float32), kr


def kernel(ptm, mem_mask):
    out, _ = run(ptm, mem_mask, trace=False)
    return out
